# revision 49
# baseline (speedup 1.0000x reference)
"""Trainium2 Bass kernel for multi-head attention (B=16, C=512, H=W=32, 8 heads).

Sharding: pure data-parallel over batch — each of the 8 NeuronCores gets 2
batches; weights are replicated. No collectives.

Per-core algorithm (per batch b):
  x[b] arrives as (C=512, S=1024) — already the transposed activation layout
  the TensorEngine wants (contraction dim on partitions).

  1. qkT = Wqk @ x[b]            -> (1024, S)   q rows 0..511, k rows 512..1023
  2. v   = x[b].T @ WvT          -> (S, 512)    (s on partitions)
     v_ext[s, st, h, 0:64] = v head h; cols 64:128 = 1.0 (wide ones block)
  3. per head h (hd=64), heads processed in pairs at partition bases 0/64 so
     their K=64 QK matmuls land in distinct PE row-groups and run concurrently:
       logitsT[kpos, q] (k on partitions, PSUM -- no transposes anywhere)
       explT = exp(0.125 * logitsT)            (ScalarE, no max subtraction --
                                                logits ~ N(0,1), max < ~6)
       po = v_ext_h.T @ explT                  -> (128, S): rows 0..63 = o^T_h,
            rows 64..127 = sum_k explT replicated 64x by the ones block, so
            the softmax denominator falls out of the same matmul and the
            reciprocal runs directly on 64 partitions (no broadcast needed)
       oT_h = po[0:64] * recip(po[64:128])     (DVE; odd head writes SBUF
                                                partitions 64..127 directly)
  4. outT = WoutT.T @ o^T  (contract c_in at K=128 over the 4 pair tiles)
     outT is (C, S) == the NCHW output layout. DMA out.

Engine budget per core (measured): PE ~190us, ACT(exp) ~147us, DVE ~95us.
Two levels of software pipelining keep both PE and ScalarE saturated:
 - pair level: pair p+1's QK+exp block is emitted BEFORE pair p's AV+normalize
   (the `pending` rotation), so ScalarE works through p+1's exps while the PE
   runs p's AV matmuls — without this the ACT-bound QK phase and the PE-only
   AV phase serialize (~50us/kernel, A/B-measured);
 - batch level: the NEXT batch's x-load + projections interleave between pairs
   (emit_proj_chunk), giving the PE dense filler work.
PSUM: three 2-bank slots are dedicated to the QK-logits -> exp pipeline
(full-width (128,1024) exps amortize the 352-cycle ACT instruction overhead);
projection and AV accumulators share two 1-bank slots (A/B-measured better
than stealing logits slots). Compute in bf16 (f32 PSUM accumulation);
tolerance 2e-2.
"""

import os

import numpy as np
import ml_dtypes

import concourse.bass as bass
from concourse import bacc
import concourse.mybir as mybir
from concourse.tile import TileContext
from concourse.bass_utils import run_bass_kernel_spmd

F32 = mybir.dt.float32
BF16 = mybir.dt.bfloat16

B, C, S = 16, 512, 1024
NH, HD = 8, 64
NCORES = 8
BPC = B // NCORES  # batches per core
KT = C // 128      # 4   k-tiles of the c_in contraction
MT_QK = 2 * C // 128  # 8 row-tiles of the qk projection output
ST = S // 128      # 8   s-tiles
NT = S // 512      # 2   512-wide chunks

LAST_EXEC_TIME_NS = None
_NC_CACHE = {}


def _build_nc(reps=1, skip_proj=False, skip_attn=False, full_exp=True, proj_pool="pso", merged=False, dma_out_psum=False, pipeline=True, po_free=False, gpx=False, ot8=False, pair_pipe=True, ex_bufs=4, ospread=False, qkt_bufs=2, psl_bufs=2, pso_bufs=4, rb_bufs=2):
    nc = bacc.Bacc(trn_type="TRN2", target_bir_lowering=False)

    x_ext = nc.declare_dram_parameter("x", [BPC, C, S], F32, isOutput=False)
    wqk_ext = nc.declare_dram_parameter("wqk_t", [C, 2 * C], BF16, isOutput=False)
    wv_ext = nc.declare_dram_parameter("wv_t", [C, C], BF16, isOutput=False)
    wo_ext = nc.declare_dram_parameter("wout_t", [C, C], BF16, isOutput=False)
    out_ext = nc.declare_dram_parameter("out", [BPC, C, S], F32, isOutput=True)

    with TileContext(nc) as tc:
        with (
            tc.tile_pool(name="const", bufs=1) as const,
            tc.tile_pool(name="acts", bufs=2) as acts,
            tc.tile_pool(name="expl", bufs=ex_bufs) as expl_pool,
            tc.tile_pool(name="oT", bufs=(8 if (ot8 or ospread) else 6)) as oT_pool,
            tc.tile_pool(name="rc", bufs=2) as rc_pool,
            tc.tile_pool(name="osb", bufs=2) as osb_pool,
            tc.tile_pool(name="psl", bufs=(psl_bufs or (4 if merged else (3 if full_exp else 6))), space="PSUM") as psl,
            tc.tile_pool(name="pso", bufs=(pso_bufs or (1 if merged else 2)), space="PSUM") as pso,
        ):
            # ---- weights (bf16 straight from HBM) ----
            wqk_bf = const.tile([128, KT, 2 * C], BF16, name="wqk_bf")
            nc.sync.dma_start(
                out=wqk_bf, in_=wqk_ext[:, :].rearrange("(kt p) n -> p kt n", p=128)
            )
            wv_bf = const.tile([128, KT, C], BF16, name="wv_bf")
            nc.sync.dma_start(
                out=wv_bf, in_=wv_ext[:, :].rearrange("(kt p) n -> p kt n", p=128)
            )
            wo_bf = const.tile([128, KT, C], BF16, name="wo_bf")
            nc.sync.dma_start(
                out=wo_bf, in_=wo_ext[:, :].rearrange("(kt p) n -> p kt n", p=128)
            )
            # v_ext[s, st, h, 0:64] = v head h; cols 64:128 stay 1.0 so the AV
            # matmul replicates the softmax denominator into rows 64:128.
            v_ext_tiles = []
            for i in range(2):
                v_ext = const.tile([128, ST, NH, 128], BF16, name=f"v_ext{i}")
                nc.vector.memset(v_ext[:, :, :, HD:], 1.0)
                if skip_proj:
                    nc.vector.memset(v_ext[:, :, :, 0:HD], 0.01)
                v_ext_tiles.append(v_ext)

            # ---- software pipeline: next batch's load+projections interleave
            # with this batch's attention pairs so ACT never drains ----
            seq = [i % BPC for i in range(reps * BPC)]
            state = {}

            def emit_load(i):
                b = seq[i]
                xb = acts.tile([128, KT, S], BF16, tag="xb", name="xb")
                x_dram = x_ext[b, :, :].rearrange("(ct p) s -> p ct s", p=128)
                for ct in range(KT):
                    x_f = acts.tile([128, S], F32, tag="xf", name="x_f", bufs=2)
                    nc.sync.dma_start(out=x_f, in_=x_dram[:, ct, :])
                    if gpx:
                        nc.gpsimd.tensor_copy(out=xb[:, ct, :], in_=x_f)
                    else:
                        nc.vector.tensor_copy(out=xb[:, ct, :], in_=x_f)
                qkT = acts.tile([128, MT_QK, S], BF16, tag="qkT", name="qkT", bufs=qkt_bufs)
                if skip_proj:
                    nc.vector.memset(qkT, 0.02)
                state[i] = {"xb": xb, "qkT": qkT, "v_ext": v_ext_tiles[i % 2]}

            def emit_proj_chunk(i, q):
                if skip_proj:
                    return
                s = state[i]
                xb, qkT, v_ext = s["xb"], s["qkT"], s["v_ext"]
                if q < 2:
                    for mt in range(4 * q, 4 * q + 4):
                        for nt in range(NT):
                            nsl = slice(nt * 512, (nt + 1) * 512)
                            pp = pso if proj_pool == "pso" else psl
                            ps = pp.tile([128, 512], F32, tag=("po" if proj_pool == "pso" else "ps"), name="ps_qk")
                            for kt in range(KT):
                                nc.tensor.matmul(
                                    ps,
                                    lhsT=wqk_bf[:, kt, mt * 128:(mt + 1) * 128],
                                    rhs=xb[:, kt, nsl],
                                    start=(kt == 0),
                                    stop=(kt == KT - 1),
                                )
                            nc.vector.tensor_copy(out=qkT[:, mt, nsl], in_=ps)
                else:
                    for st in range(4 * (q - 2), 4 * (q - 2) + 4):
                        pp = pso if proj_pool == "pso" else psl
                        ps = pp.tile([128, C], F32, tag=("po" if proj_pool == "pso" else "ps"), name="ps_v")
                        for kt in range(KT):
                            nc.tensor.matmul(
                                ps,
                                lhsT=xb[:, kt, st * 128:(st + 1) * 128],
                                rhs=wv_bf[:, kt, :],
                                start=(kt == 0),
                                stop=(kt == KT - 1),
                            )
                        nc.vector.tensor_copy(
                            out=v_ext[:, st, :, 0:HD],
                            in_=ps.rearrange("p (h d) -> p h d", h=NH),
                        )

            def emit_pair_qk(i, hp):
                s = state[i]
                qkT = s["qkT"]
                qA = qkT[0:64, hp, :]
                kA = qkT[0:64, NH // 2 + hp, :]
                qB = qkT[64:128, hp, :]
                kB = qkT[64:128, NH // 2 + hp, :]

                exA = expl_pool.tile([128, ST, S], BF16, tag="ex", name="exA")
                exB = expl_pool.tile([128, ST, S], BF16, tag="ex", name="exB")
                for kt in range(ST):
                    ksl = slice(kt * 128, (kt + 1) * 128)
                    if full_exp:
                        psA = psl.tile([128, S], F32, tag="ps", name="ps_lA")
                        psB = psl.tile([128, S], F32, tag="ps", name="ps_lB")
                        for nt in range(NT):
                            nsl = slice(nt * 512, (nt + 1) * 512)
                            nc.tensor.matmul(psA[:, nsl], lhsT=kA[:, ksl],
                                             rhs=qA[:, nsl], start=True, stop=True)
                            nc.tensor.matmul(psB[:, nsl], lhsT=kB[:, ksl],
                                             rhs=qB[:, nsl], start=True, stop=True)
                        nc.scalar.activation(
                            out=exA[:, kt, :], in_=psA,
                            func=mybir.ActivationFunctionType.Exp, scale=0.125)
                        nc.scalar.activation(
                            out=exB[:, kt, :], in_=psB,
                            func=mybir.ActivationFunctionType.Exp, scale=0.125)
                    else:
                        for nt in range(NT):
                            nsl = slice(nt * 512, (nt + 1) * 512)
                            psA = psl.tile([128, 512], F32, tag="ps", name="ps_lA")
                            psB = psl.tile([128, 512], F32, tag="ps", name="ps_lB")
                            nc.tensor.matmul(psA, lhsT=kA[:, ksl],
                                             rhs=qA[:, nsl], start=True, stop=True)
                            nc.tensor.matmul(psB, lhsT=kB[:, ksl],
                                             rhs=qB[:, nsl], start=True, stop=True)
                            nc.scalar.activation(
                                out=exA[:, kt, nsl], in_=psA,
                                func=mybir.ActivationFunctionType.Exp, scale=0.125)
                            nc.scalar.activation(
                                out=exB[:, kt, nsl], in_=psB,
                                func=mybir.ActivationFunctionType.Exp, scale=0.125)
                return exA, exB

            def emit_pair_av(i, hp, exA, exB):
                s = state[i]
                v_ext = s["v_ext"]
                hA, hB = 2 * hp, 2 * hp + 1
                oT2 = oT_pool.tile([128, S], BF16, tag="oT", name="oT2")
                for idx, (h, ex) in enumerate(((hA, exA), (hB, exB))):
                    for nt in range(NT):
                        nsl = slice(nt * 512, (nt + 1) * 512)
                        po = (psl if merged else pso).tile([128, 512], F32, tag=("ps" if merged else "po"), name="po")
                        for kt in range(ST):
                            nc.tensor.matmul(
                                po,
                                lhsT=v_ext[:, kt, h, :],
                                rhs=ex[:, kt, nsl],
                                start=(kt == 0),
                                stop=(kt == ST - 1),
                            )
                        rb = rc_pool.tile([HD, 512], BF16, tag="rb",
                                          name="rb", bufs=rb_bufs)
                        with nc.allow_low_precision(reason="bf16 denom"):
                            nc.vector.reciprocal(rb, po[HD:, :])
                        nc.vector.tensor_mul(
                            oT2[idx * HD:(idx + 1) * HD, nsl], po[0:HD, :], rb
                        )
                s.setdefault("oT", []).append(oT2)

            def emit_pair(i, hp):
                if skip_attn:
                    oT2 = oT_pool.tile([128, S], BF16, tag="oT", name="oT2")
                    nc.vector.tensor_copy(out=oT2, in_=state[i]["qkT"][:, hp, :])
                    state[i].setdefault("oT", []).append(oT2)
                    return
                exA, exB = emit_pair_qk(i, hp)
                emit_pair_av(i, hp, exA, exB)

            def emit_outproj_mt(i, mt):
                b = seq[i]
                oT_tiles = state[i]["oT"]
                out_dram = out_ext[b, :, :].rearrange("(mt p) s -> p mt s", p=128)
                if True:
                    out_sb = None
                    if not dma_out_psum:
                        out_sb = osb_pool.tile([128, S], F32, tag="osb", name="out_sb")
                    for nt in range(NT):
                        nsl = slice(nt * 512, (nt + 1) * 512)
                        pp = pso if proj_pool == "pso" else psl
                        ps = pp.tile([128, 512], F32, tag=("po" if proj_pool == "pso" else "ps"), name="ps_o")
                        for j in range(KT):
                            nc.tensor.matmul(
                                ps,
                                lhsT=wo_bf[:, j, mt * 128:(mt + 1) * 128],
                                rhs=oT_tiles[j][:, nsl],
                                start=(j == 0),
                                stop=(j == KT - 1),
                            )
                        if dma_out_psum:
                            nc.sync.dma_start(out=out_dram[:, mt, nsl], in_=ps)
                        else:
                            nc.vector.tensor_copy(out=out_sb[:, nsl], in_=ps)
                    if not dma_out_psum:
                        nc.sync.dma_start(out=out_dram[:, mt, :], in_=out_sb)

            def emit_outproj(i):
                for mt in range(KT):
                    emit_outproj_mt(i, mt)
                del state[i]

            if pipeline and pair_pipe and not skip_attn:
                # pair-level software pipeline: pair p+1's QK+exp is emitted
                # BEFORE pair p's AV so ScalarE (exp) stays busy while the PE
                # runs AV, and vice versa.
                emit_load(0)
                for q in range(4):
                    emit_proj_chunk(0, q)
                pending = None
                odone = []  # batches whose outproj chunks remain, with next mt
                for i in range(len(seq)):
                    if i + 1 < len(seq):
                        emit_load(i + 1)
                    for hp in range(NH // 2):
                        exA, exB = emit_pair_qk(i, hp)
                        if pending is not None:
                            pi, php, pA, pB = pending
                            emit_pair_av(pi, php, pA, pB)
                            if php == NH // 2 - 1:
                                if ospread:
                                    odone.append([pi, 0])
                                else:
                                    emit_outproj(pi)
                        pending = (i, hp, exA, exB)
                        if ospread and odone:
                            pi2, mt = odone[0]
                            emit_outproj_mt(pi2, mt)
                            if mt == KT - 1:
                                del state[pi2]
                                odone.pop(0)
                            else:
                                odone[0][1] += 1
                        if i + 1 < len(seq):
                            emit_proj_chunk(i + 1, hp)
                pi, php, pA, pB = pending
                emit_pair_av(pi, php, pA, pB)
                if ospread:
                    for pi2, mt0 in odone:
                        for mt in range(mt0, KT):
                            emit_outproj_mt(pi2, mt)
                        del state[pi2]
                    emit_outproj(pi)
                else:
                    emit_outproj(pi)
            elif pipeline:
                emit_load(0)
                for q in range(4):
                    emit_proj_chunk(0, q)
                for i in range(len(seq)):
                    if i + 1 < len(seq):
                        emit_load(i + 1)
                    for hp in range(NH // 2):
                        emit_pair(i, hp)
                        if i + 1 < len(seq):
                            emit_proj_chunk(i + 1, hp)
                    emit_outproj(i)
            else:
                for i in range(len(seq)):
                    emit_load(i)
                    for q in range(4):
                        emit_proj_chunk(i, q)
                    for hp in range(NH // 2):
                        emit_pair(i, hp)
                    emit_outproj(i)

    nc.compile()
    return nc


def _get_nc(reps=1):
    if reps not in _NC_CACHE:
        _NC_CACHE[reps] = _build_nc(reps)
    return _NC_CACHE[reps]


def kernel(x, w_qkv, w_out):
    global LAST_EXEC_TIME_NS
    x = np.ascontiguousarray(np.asarray(x, dtype=np.float32)).reshape(B, C, S)
    w_qkv = np.asarray(w_qkv, dtype=np.float32)
    w_out = np.asarray(w_out, dtype=np.float32)

    wqk_t = np.ascontiguousarray(w_qkv[: 2 * C].T).astype(ml_dtypes.bfloat16)
    wv_t = np.ascontiguousarray(w_qkv[2 * C:].T).astype(ml_dtypes.bfloat16)
    wout_t = np.ascontiguousarray(w_out.T).astype(ml_dtypes.bfloat16)

    # this trimmed container lacks the NTFF profile hook (antenv.axon_hooks);
    # make sure an inherited BASS_TRACE can't route us into that import.
    os.environ["BASS_NEVER_TRACE"] = "1"
    nc = _get_nc()
    in_maps = [
        {
            "x": x[i * BPC:(i + 1) * BPC],
            "wqk_t": wqk_t,
            "wv_t": wv_t,
            "wout_t": wout_t,
        }
        for i in range(NCORES)
    ]
    res = run_bass_kernel_spmd(nc, in_maps, core_ids=list(range(NCORES)))
    LAST_EXEC_TIME_NS = res.exec_time_ns
    out = np.concatenate([res.results[i]["out"] for i in range(NCORES)], axis=0)
    return out.reshape(B, C, 32, 32)


if __name__ == "__main__":
    _build_nc()
    print("build OK")
